# revision 23
# baseline (speedup 1.0000x reference)
"""ConceptCLIP loss kernel for 8x Trainium2 NeuronCores (Bass/Tile).

Strategy (data-parallel over the image batch axis m):
  - Each core owns 16 of the 128 images; concept/text features are replicated.
  - Concepts are host-packed (only w < counts[v] kept) and quantized to
    fp8e4m3 RAW; their 1/||c|| is folded into the G gather matrix on device.
  - Patches are host-TRANSPOSED to (d, n) layout (pure layout, free) and
    shipped bf16 twice (transposed for matmul, natural for norms). The
    per-patch 16/||x|| scale comes from ACT square+accum on the natural copy,
    is moved to row form by tiny DMAs, broadcast across partitions by GPSIMD,
    and applied by DVE with fp8e4m3 output (single quantization).
  - Main loop: fp8 DoubleRow matmuls (contraction 256/instr, 2x ALU rate):
    per (c-chunk, half) one [128,4,512] PSUM tile holds 4 image pairs; 12
    matmuls fill it; the drain is a tensor_tensor MAX fold (2 PSUM reads per
    cycle, bf16 out) + a cheap bf16 reduce_max -> maxcol. Folds alternate
    between GPSIMD and DVE to keep DVE off the critical path.
  - S = (G*rnorm/16)^T @ maxcol in bf16; IT-align logits via raw bf16 matmul
    with deferred rank-1 normalization. Softplus loss elements are DMA'd out;
    host sums them.
"""

import math
import os
import sys

for _p in ("/opt/trn_rl_repo", "/root/.axon_site/_ro/trn_rl_repo"):
    if os.path.isdir(_p) and _p not in sys.path:
        sys.path.insert(0, _p)

import ml_dtypes
import numpy as np

import concourse.tile as tile
from concourse import bacc, mybir
from concourse.bass_utils import run_bass_kernel_spmd

BF16 = ml_dtypes.bfloat16
FP8 = ml_dtypes.float8_e4m3

N_CORES = 8
B, NPATCH, D, W = 128, 196, 768, 32
M_PER = B // N_CORES   # 16 images per core
PAIRS = M_PER // 2     # 8 image pairs
KC = D // 128          # 6 contraction chunks of 128
NKP = KC // 2          # 3 DoubleRow k-pairs (contraction 256 each)
VCOLS = 2 * NPATCH     # 392 valid columns per pair (imgA 0:196, imgB 196:392)
COLS = 400             # padded to 16B-aligned k-chunk stride

F32 = mybir.dt.float32
BF = mybir.dt.bfloat16
F8 = mybir.dt.float8e4
AX = mybir.AxisListType
AF = mybir.ActivationFunctionType
ALU = mybir.AluOpType
DR = mybir.MatmulPerfMode.DoubleRow

_cache = {}


def _build(C, t, bias):
    """Build + compile the per-core Bass program. C = number of 128-row packed
    concept chunks; t/bias are compile-time scalar constants."""
    P2 = C * 128
    nc = bacc.Bacc("TRN2", target_bir_lowering=False, debug=False,
                   num_devices=N_CORES)

    d_pT = nc.dram_tensor("pT", (PAIRS, 128, KC, COLS), BF, kind="ExternalInput")
    d_pnat = nc.dram_tensor("pnat", (M_PER, NPATCH, D), BF, kind="ExternalInput")
    d_cT = nc.dram_tensor("cT", (NKP, 128, 2, P2), F8, kind="ExternalInput")
    d_cnat = nc.dram_tensor("cnat", (P2, D), BF, kind="ExternalInput")
    d_GT = nc.dram_tensor("GT", (C, 128, B), BF, kind="ExternalInput")
    d_txtT = nc.dram_tensor("txtT", (128, KC, 128), BF, kind="ExternalInput")
    d_imgT = nc.dram_tensor("imgT", (128, KC, M_PER), BF, kind="ExternalInput")
    d_txtn = nc.dram_tensor("txtn", (B, D), BF, kind="ExternalInput")
    d_sign = nc.dram_tensor("signneg", (B, M_PER), F32, kind="ExternalInput")
    d_rc = nc.dram_tensor("rc_el", (B, M_PER), F32, kind="ExternalOutput")
    d_it = nc.dram_tensor("it_el", (B, M_PER), F32, kind="ExternalOutput")

    with tile.TileContext(nc) as tc:
        with (
            tc.tile_pool(name="consts", bufs=1) as consts,
            tc.tile_pool(name="work", bufs=3) as work,
            tc.tile_pool(name="small", bufs=4) as small,
            tc.tile_pool(name="psum", bufs=2, space="PSUM") as psum,
            tc.tile_pool(name="dscr", bufs=2, space="DRAM") as dscr,
        ):
            sign = consts.tile([B, M_PER], F32, tag="sign")
            nc.sync.dma_start(out=sign[:], in_=d_sign.ap())
            ones_col = consts.tile([128, 1], BF, tag="ones")
            nc.vector.memset(ones_col[:], 1.0)
            warm = small.tile([1, 1], F32, tag="warm")
            nc.vector.memset(warm[:], 1.0)
            nc.scalar.activation(out=warm[:], in_=warm[:], func=AF.Square)

            maxcol = consts.tile([128, C, M_PER], BF, tag="maxcol")
            rnorm = consts.tile([128, C], F32, tag="rnorm")
            css = consts.tile([128, C], F32, tag="css")
            GTbf = consts.tile([128, C * B], BF, tag="GT")
            yit = consts.tile([B, M_PER], F32, tag="yit")

            pT = [consts.tile([128, KC, COLS], BF, tag=f"pT{p}", name=f"pT{p}")
                  for p in range(PAIRS)]
            rhs8 = [consts.tile([128, KC, COLS], F8, tag=f"r8{p}", name=f"r8{p}")
                    for p in range(PAIRS)]
            cT = []
            for j in range(NKP):
                tj = consts.tile([128, 2, P2], F8, tag=f"cT{j}", name=f"cT{j}")
                nc.sync.dma_start(out=tj[:], in_=d_cT.ap()[j])
                cT.append(tj)

            PBLOCKS = ((0, 128), (128, NPATCH - 128))

            def prep_pair(pr):
                nc.sync.dma_start(out=pT[pr][:], in_=d_pT.ap()[pr])
                # ssq cols = (imgA b0, imgA b1, imgB b0, imgB b1)
                ssq = small.tile([128, 4], F32, tag="ssq", bufs=4)
                nc.vector.memset(ssq[:], 1.0)
                for i2 in range(2):
                    m = 2 * pr + i2
                    for b, (r0, nr) in enumerate(PBLOCKS):
                        natp = work.tile([128, D], BF, tag="nat", bufs=10)
                        nc.sync.dma_start(out=natp[:nr],
                                          in_=d_pnat.ap()[m, r0:r0 + nr, :])
                        scr = work.tile([128, D], BF, tag="scr", bufs=3)
                        # sum((x/16)^2) = (||x||/16)^2
                        nc.scalar.activation(out=scr[:nr], in_=natp[:nr],
                                             func=AF.Square, scale=1.0 / 16.0,
                                             accum_out=ssq[:nr, 2 * i2 + b:2 * i2 + b + 1])
                nc.scalar.sqrt(ssq[:], ssq[:])
                rb = small.tile([128, 4], BF, tag="rb", bufs=4)
                with nc.allow_low_precision(reason="scale feeds fp8 quantization"):
                    nc.vector.reciprocal(rb[:], ssq[:])   # 16/||x||, bf16
                rbd = dscr.tile([128, 4], BF, tag="rbd", bufs=2)
                nc.sync.dma_start(out=rbd[:], in_=rb[:])
                rbrow = small.tile([1, COLS], BF, tag="rbrow", bufs=2)
                nc.vector.memset(rbrow[:], 0.0)
                for i2 in range(2):
                    for b, (r0, nr) in enumerate(PBLOCKS):
                        j = 2 * i2 + b
                        off = i2 * NPATCH + r0
                        nc.sync.dma_start(
                            out=rbrow[0:1, off:off + nr],
                            in_=rbd[0:nr, j:j + 1].rearrange("a b -> b a"))
                bc = work.tile([128, COLS], BF, tag="bc", bufs=3)
                nc.gpsimd.partition_broadcast(out_ap=bc[:, :], in_ap=rbrow[0:1, :])
                # late pairs are prepped during phase A: GPSIMD has slack there
                meng = nc.vector if pr < 4 else nc.gpsimd
                for k in range(KC):
                    meng.tensor_mul(rhs8[pr][:, k, :], pT[pr][:, k, :],
                                    bc[:, :])

            for pr in range(4):
                prep_pair(pr)

            def main_phase(half, preps=()):
                # A[p_chunk, img cols]: 4 pairs per [128,4,512] PSUM tile (one
                # bank each); fp8 DoubleRow fills; MAX-fold + bf16 reduce drain.
                preps = dict(preps)
                prs = list(range(half * 4, half * 4 + 4))
                for c in range(C):
                    ps = psum.tile([128, 4, 512], F32, tag="mm", bufs=2,
                                   name="mm")
                    for j in range(NKP):
                        for i, pr in enumerate(prs):
                            nc.tensor.matmul(ps[:, i, 0:COLS],
                                             lhsT=cT[j][:, :, c * 128:(c + 1) * 128],
                                             rhs=rhs8[pr][:, 2 * j:2 * j + 2, :],
                                             start=(j == 0), stop=(j == NKP - 1),
                                             perf_mode=DR)
                    out_ap = maxcol[:, c, 8 * half:8 * half + 8].rearrange(
                        "p (b s) -> p b s", s=2)
                    v = ps[:, :, 0:VCOLS].rearrange("p b (s x) -> p b s x", s=2)
                    if c % 2 == 1:
                        # bounce through ACT (it can read PSUM) to offload DVE
                        cps = work.tile([128, 4, 2, NPATCH], BF, tag="cps",
                                        bufs=2)
                        nc.scalar.copy(out=cps[:], in_=v)
                        nc.vector.reduce_max(out=out_ap, in_=cps[:], axis=AX.X)
                    else:
                        nc.vector.reduce_max(out=out_ap, in_=v, axis=AX.X)
                    if c in preps:
                        prep_pair(preps[c])

            main_phase(0, preps={1: 4, 4: 5, 7: 6, 10: 7})

            # ---- concept row norms + G scaling (overlaps with main loop) ----
            for c in range(C):
                cn = work.tile([128, D], BF, tag="cnat", bufs=3)
                nc.sync.dma_start(out=cn[:], in_=d_cnat.ap()[c * 128:(c + 1) * 128, :])
                scr = work.tile([128, D], BF, tag="scr", bufs=3)
                nc.scalar.activation(out=scr[:], in_=cn[:], func=AF.Square,
                                     accum_out=css[:, c:c + 1])
            nc.scalar.sqrt(css[:], css[:])
            nc.vector.reciprocal(rnorm[:], css[:])
            for c in range(C):
                nc.sync.dma_start(out=GTbf[:, c * B:(c + 1) * B], in_=d_GT.ap()[c])
                nc.vector.tensor_scalar_mul(GTbf[:, c * B:(c + 1) * B],
                                            GTbf[:, c * B:(c + 1) * B],
                                            rnorm[:, c:c + 1])

            # ---- IT-align: raw bf16 matmul + deferred rank-1 normalization --
            txtT = consts.tile([128, KC, 128], BF, tag="txtT")
            nc.sync.dma_start(out=txtT[:], in_=d_txtT.ap())
            imgT = consts.tile([128, KC, M_PER], BF, tag="imgT")
            nc.sync.dma_start(out=imgT[:], in_=d_imgT.ap())
            txtn = work.tile([128, D], BF, tag="nat", bufs=10)
            nc.sync.dma_start(out=txtn[:], in_=d_txtn.ap())
            tscr = work.tile([128, D], BF, tag="scr", bufs=3)
            tss = small.tile([128, 1], F32, tag="tss")
            nc.scalar.activation(out=tscr[:], in_=txtn[:], func=AF.Square,
                                 accum_out=tss[:])
            nc.scalar.sqrt(tss[:], tss[:])
            av = small.tile([128, 1], F32, tag="av")
            nc.vector.reciprocal(av[:], tss[:])
            nc.vector.tensor_scalar_mul(av[:], av[:], float(t))  # t/||txt_v||

            # img norms via ones-matmul on squared imgT (transposed layout)
            isq = small.tile([128, KC, M_PER], BF, tag="isq")
            nc.scalar.activation(out=isq[:], in_=imgT[:], func=AF.Square)
            aux = psum.tile([128, 4, 512], F32, tag="mm", bufs=2, name="aux")
            for k in range(KC):
                nc.tensor.matmul(aux[0:1, 0, 0:M_PER], lhsT=ones_col[:, :],
                                 rhs=isq[:, k, :], start=(k == 0),
                                 stop=(k == KC - 1))
            ib = small.tile([1, M_PER], F32, tag="ib")
            nc.scalar.sqrt(ib[:], aux[0:1, 0, 0:M_PER])
            nc.vector.reciprocal(ib[:], ib[:])                   # 1/||img_m||
            bg = small.tile([128, M_PER], F32, tag="bg")
            nc.gpsimd.partition_broadcast(out_ap=bg[:, :], in_ap=ib[0:1, :])

            for k in range(KC):
                nc.tensor.matmul(aux[:, 1, 0:M_PER], lhsT=txtT[:, k, :],
                                 rhs=imgT[:, k, :], start=(k == 0),
                                 stop=(k == KC - 1))
            nc.scalar.activation(out=yit[:], in_=aux[:, 1, 0:M_PER], func=AF.Copy,
                                 scale=av[:])
            nc.vector.tensor_mul(yit[:], yit[:], bg[:])
            nc.vector.tensor_scalar_add(yit[:], yit[:], float(bias))

            nc.scalar.activation(out=warm[:], in_=warm[:], func=AF.Exp)

            main_phase(1)

            # ---- S[v, m] = sum_p Geff[p,v] * maxcol[p,m]  (bf16, fp32 acc) --
            sps = psum.tile([128, 4, 512], F32, tag="mm", bufs=2, name="sps")
            for c in range(C):
                nc.tensor.matmul(sps[:, 0, 0:M_PER], lhsT=GTbf[:, c * B:(c + 1) * B],
                                 rhs=maxcol[:, c, :], start=(c == 0),
                                 stop=(c == C - 1))

            def softplus_out(y_ap, d_out):
                el = small.tile([B, M_PER], F32, tag="el", name="el")
                nc.scalar.activation(out=el[:], in_=y_ap, func=AF.Exp)
                nc.vector.tensor_scalar_add(el[:], el[:], 1.0)
                nc.scalar.activation(out=el[:], in_=el[:], func=AF.Ln)
                nc.sync.dma_start(out=d_out.ap(), in_=el[:])

            yrc = small.tile([B, M_PER], F32, tag="y")
            nc.scalar.activation(out=yrc[:], in_=sps[:, 0, 0:M_PER], func=AF.Copy,
                                 bias=float(bias), scale=float(t))
            nc.vector.tensor_mul(yrc[:], yrc[:], sign[:])
            softplus_out(yrc[:], d_rc)

            nc.vector.tensor_mul(yit[:], yit[:], sign[:])
            softplus_out(yit[:], d_it)

    nc.compile()
    return nc


def _install_trace_hook():
    """Register the axon NTFF profiling hook (missing from this image) so
    run_bass_kernel_spmd(trace=True) can capture HW exec time."""
    import contextlib
    import ctypes
    import types

    import concourse.bass_utils as bu

    if "antenv.axon_hooks" in sys.modules:
        return
    so_path = "/opt/axon/libaxon_pjrt.so"

    def _make_hook():
        lib = ctypes.CDLL(so_path)
        if not hasattr(lib, "axon_start_nrt_profile"):
            return None
        lib.axon_start_nrt_profile.argtypes = [ctypes.POINTER(ctypes.c_int64),
                                               ctypes.c_size_t]
        lib.axon_start_nrt_profile.restype = ctypes.c_int64
        lib.axon_stop_nrt_profile.argtypes = [ctypes.c_char_p]
        lib.axon_stop_nrt_profile.restype = ctypes.c_int64

        @contextlib.contextmanager
        def _hook(output_dir, device_ids):
            import jax
            jax.devices()
            if device_ids:
                ids = (ctypes.c_int64 * len(device_ids))(*device_ids)
                rc = lib.axon_start_nrt_profile(ids, len(device_ids))
            else:
                rc = lib.axon_start_nrt_profile(None, 0)
            if rc != 0:
                raise RuntimeError(f"axon_start_nrt_profile rc={rc}")
            try:
                yield
            finally:
                n = lib.axon_stop_nrt_profile(str(output_dir).encode())
                print(f"profile: {n} file(s) written to {output_dir}",
                      file=sys.stderr)

        return _hook

    mod = types.ModuleType("antenv.axon_hooks")
    mod.get_axon_ntff_profile_hook = _make_hook
    sys.modules["antenv.axon_hooks"] = mod
    bu.upload_artifacts = lambda tmpdir: tmpdir  # no S3 in this container


def _prepare(inputs):
    image_features = np.asarray(inputs["image_features"], np.float32)
    text_features = np.asarray(inputs["text_features"], np.float32)
    image_token_features = np.asarray(inputs["image_token_features"], np.float32)
    concept_text_features = np.asarray(inputs["concept_text_features"], np.float32)
    counts = np.asarray(inputs["concept_counts"]).astype(np.int64)
    t = float(np.exp(np.clip(np.float32(inputs["logit_scale"]), -10.0, 10.0)))
    bias = float(np.float32(inputs["logit_bias"]))

    # pack concepts: keep only w < counts[v]; pad rows with ones (zero weight)
    vidx = np.repeat(np.arange(B), counts)
    widx = np.concatenate([np.arange(c) for c in counts])
    P = len(vidx)
    C = math.ceil(P / 128)
    P2 = C * 128
    cnat = np.ones((P2, D), np.float32)
    cnat[:P] = concept_text_features[vidx, widx]
    cnat_bf = cnat.astype(BF16)
    # cT[j, d128, h, p] = fp8(cnat[p, (2j+h)*128 + d])
    cT = np.ascontiguousarray(
        cnat.astype(FP8).T.reshape(NKP, 2, 128, P2).transpose(0, 2, 1, 3))

    # G with 1/(16*counts): folds away the x16 patch scale
    G = np.zeros((P2, B), np.float32)
    G[np.arange(P), vidx] = 1.0 / (16.0 * counts[vidx])
    GT = G.astype(BF16).reshape(C, 128, B)

    txt_bf = text_features.astype(BF16)
    # txtT[d, k, v] = txt_bf[v, k*128 + d]
    txtT = np.ascontiguousarray(
        txt_bf.T.reshape(KC, 128, B).transpose(1, 0, 2))

    in_maps = []
    for core in range(N_CORES):
        s = slice(core * M_PER, (core + 1) * M_PER)
        sh = image_token_features[s].astype(BF16)        # (16, 196, 768)
        pT = np.zeros((PAIRS, 128, KC, COLS), BF16)
        shT = sh.transpose(0, 2, 1).reshape(M_PER, KC, 128, NPATCH)
        # pT[pr, d, k, 196*i2 + n] = patches[2pr+i2][n, k*128+d]
        pT[:, :, :, 0:NPATCH] = shT[0::2].transpose(0, 2, 1, 3)
        pT[:, :, :, NPATCH:VCOLS] = shT[1::2].transpose(0, 2, 1, 3)
        img_bf = image_features[s].astype(BF16)          # (16, 768)
        imgT = np.ascontiguousarray(
            img_bf.T.reshape(KC, 128, M_PER).transpose(1, 0, 2))
        signneg = np.ones((B, M_PER), np.float32)
        for j in range(M_PER):
            signneg[core * M_PER + j, j] = -1.0
        in_maps.append({
            "pT": pT,
            "pnat": sh,
            "cT": cT,
            "cnat": cnat_bf,
            "GT": GT,
            "txtT": txtT,
            "imgT": imgT,
            "txtn": txt_bf,
            "signneg": signneg,
        })
    return in_maps, C, t, bias


def _run(inputs, trace=False, tmpdir=None):
    in_maps, C, t, bias = _prepare(inputs)
    key = (C, t, bias)
    if key not in _cache:
        _cache[key] = _build(C, t, bias)
    nc = _cache[key]
    kwargs = {}
    if trace:
        _install_trace_hook()
        kwargs = dict(trace=True, tmpdir=tmpdir)
    res = run_bass_kernel_spmd(nc, in_maps, core_ids=list(range(N_CORES)),
                               **kwargs)
    it_sum = sum(float(r["it_el"].astype(np.float64).sum()) for r in res.results)
    rc_sum = sum(float(r["rc_el"].astype(np.float64).sum()) for r in res.results)
    it_loss = it_sum / (B * B)
    rc_loss = rc_sum / (B * B)
    total = it_loss + 0.5 * rc_loss
    out = (np.float32(total), np.float32(it_loss), np.float32(rc_loss))
    return out, res


def kernel(**inputs):
    out, _ = _run(inputs)
    return out


# revision 24
# speedup vs baseline: 1.0408x; 1.0408x over previous
"""ConceptCLIP loss kernel for 8x Trainium2 NeuronCores (Bass/Tile).

Strategy (data-parallel over the image batch axis m):
  - Each core owns 16 of the 128 images; concept/text features are replicated.
  - Concepts are host-packed (only w < counts[v] kept) and quantized to
    fp8e4m3 RAW; their 1/||c|| is folded into the G gather matrix on device.
  - Patches are host-TRANSPOSED to (d, n) layout (pure layout, free) and
    shipped bf16. Per-patch sum-of-squares comes from squaring (ACT/GPSIMD
    split) + a (1/256)-vector matmul on PE, giving (||x||/16)^2 directly in
    row form; sqrt -> reciprocal -> GPSIMD partition_broadcast -> multiply
    (DVE for early pairs, GPSIMD for late ones) yields fp8 operands holding
    16*x/||x|| (single quantization).
  - Main loop: fp8 DoubleRow matmuls (contraction 256/instr, 2x ALU rate) in
    4 phases of 2 image pairs; each (c, phase) fills a [128,2,512] PSUM tile
    (1 bank per pair, 392 valid cols). Drains alternate between a direct DVE
    reduce_max and an ACT copy->SBUF-bf16 + cheap DVE reduce to split the
    PSUM-read load across both engines.
  - S = (G*rnorm/16)^T @ maxcol in bf16; IT-align logits via raw bf16 matmul
    with deferred rank-1 normalization. Softplus loss elements are DMA'd out;
    host sums them.
"""

import math
import os
import sys

for _p in ("/opt/trn_rl_repo", "/root/.axon_site/_ro/trn_rl_repo"):
    if os.path.isdir(_p) and _p not in sys.path:
        sys.path.insert(0, _p)

import ml_dtypes
import numpy as np

import concourse.tile as tile
from concourse import bacc, mybir
from concourse.bass_utils import run_bass_kernel_spmd

BF16 = ml_dtypes.bfloat16
FP8 = ml_dtypes.float8_e4m3

N_CORES = 8
B, NPATCH, D, W = 128, 196, 768, 32
M_PER = B // N_CORES   # 16 images per core
PAIRS = M_PER // 2     # 8 image pairs
KC = D // 128          # 6 contraction chunks of 128
NKP = KC // 2          # 3 DoubleRow k-pairs (contraction 256 each)
VCOLS = 2 * NPATCH     # 392 valid columns per pair (imgA 0:196, imgB 196:392)
COLS = 400             # padded to 16B-aligned k-chunk stride

F32 = mybir.dt.float32
BF = mybir.dt.bfloat16
F8 = mybir.dt.float8e4
AX = mybir.AxisListType
AF = mybir.ActivationFunctionType
DR = mybir.MatmulPerfMode.DoubleRow

_cache = {}


def _build(C, t, bias):
    """Build + compile the per-core Bass program. C = number of 128-row packed
    concept chunks; t/bias are compile-time scalar constants."""
    P2 = C * 128
    nc = bacc.Bacc("TRN2", target_bir_lowering=False, debug=False,
                   num_devices=N_CORES)

    d_pT = nc.dram_tensor("pT", (PAIRS, 128, KC, COLS), BF, kind="ExternalInput")
    d_cT = nc.dram_tensor("cT", (NKP, 128, 2, P2), F8, kind="ExternalInput")
    d_cnat = nc.dram_tensor("cnat", (P2, D), BF, kind="ExternalInput")
    d_GT = nc.dram_tensor("GT", (C, 128, B), BF, kind="ExternalInput")
    d_txtT = nc.dram_tensor("txtT", (128, KC, 128), BF, kind="ExternalInput")
    d_imgT = nc.dram_tensor("imgT", (128, KC, M_PER), BF, kind="ExternalInput")
    d_txtn = nc.dram_tensor("txtn", (B, D), BF, kind="ExternalInput")
    d_sign = nc.dram_tensor("signneg", (B, M_PER), F32, kind="ExternalInput")
    d_rc = nc.dram_tensor("rc_el", (B, M_PER), F32, kind="ExternalOutput")
    d_it = nc.dram_tensor("it_el", (B, M_PER), F32, kind="ExternalOutput")

    with tile.TileContext(nc) as tc:
        with (
            tc.tile_pool(name="consts", bufs=1) as consts,
            tc.tile_pool(name="work", bufs=3) as work,
            tc.tile_pool(name="small", bufs=4) as small,
            tc.tile_pool(name="psum", bufs=2, space="PSUM") as psum,
        ):
            sign = consts.tile([B, M_PER], F32, tag="sign")
            nc.sync.dma_start(out=sign[:], in_=d_sign.ap())
            ones_col = consts.tile([128, 1], BF, tag="ones")
            nc.vector.memset(ones_col[:], 1.0)
            sc_col = consts.tile([128, 1], BF, tag="sc")
            nc.vector.memset(sc_col[:], 1.0 / 256.0)
            warm = small.tile([1, 1], F32, tag="warm")
            nc.vector.memset(warm[:], 1.0)
            nc.scalar.activation(out=warm[:], in_=warm[:], func=AF.Square)

            maxcol = consts.tile([128, C, M_PER], BF, tag="maxcol")
            rnorm = consts.tile([128, C], F32, tag="rnorm")
            css = consts.tile([128, C], F32, tag="css")
            GTbf = consts.tile([128, C * B], BF, tag="GT")
            yit = consts.tile([B, M_PER], F32, tag="yit")

            pT = [consts.tile([128, KC, COLS], BF, tag=f"pT{p}", name=f"pT{p}")
                  for p in range(PAIRS)]
            rhs8 = [consts.tile([128, KC, COLS], F8, tag=f"r8{p}", name=f"r8{p}")
                    for p in range(PAIRS)]
            cT = []
            for j in range(NKP):
                tj = consts.tile([128, 2, P2], F8, tag=f"cT{j}", name=f"cT{j}")
                nc.sync.dma_start(out=tj[:], in_=d_cT.ap()[j])
                cT.append(tj)

            sq_tiles = {}

            def prep_squares(pr):
                # sq = (pT)^2 in bf16; k<4 on ACT, k>=4 on GPSIMD
                nc.sync.dma_start(out=pT[pr][:], in_=d_pT.ap()[pr])
                sq = work.tile([128, KC, COLS], BF, tag="sq", bufs=3)
                for k in range(KC):
                    if k < 4:
                        nc.scalar.activation(out=sq[:, k, :], in_=pT[pr][:, k, :],
                                             func=AF.Square)
                    else:
                        nc.gpsimd.tensor_mul(sq[:, k, :], pT[pr][:, k, :],
                                             pT[pr][:, k, :])
                sq_tiles[pr] = sq

            def prep_finish(pr2):
                # (1/256)-matmul -> (||x||/16)^2 row per pair; sqrt; recip;
                # broadcast; scale to fp8.  pr2 = (prA, prB) sharing one tile.
                bt = psum.tile([128, 2, 512], F32, tag="aux", bufs=2, name="bt")
                for i, pr in enumerate(pr2):
                    for k in range(KC):
                        nc.tensor.matmul(bt[0:1, i, 0:COLS], lhsT=sc_col[:, :],
                                         rhs=sq_tiles[pr][:, k, :],
                                         start=(k == 0), stop=(k == KC - 1))
                for i, pr in enumerate(pr2):
                    del sq_tiles[pr]
                    srow = small.tile([1, COLS], F32, tag="srow", bufs=2)
                    nc.scalar.sqrt(srow[:], bt[0:1, i, 0:COLS])
                    rrec = small.tile([1, COLS], BF, tag="rrec", bufs=2)
                    with nc.allow_low_precision(reason="feeds fp8 quantization"):
                        nc.vector.reciprocal(rrec[:], srow[:])  # 16/||x||
                    bc = work.tile([128, COLS], BF, tag="bc", bufs=3)
                    nc.gpsimd.partition_broadcast(out_ap=bc[:, :],
                                                  in_ap=rrec[0:1, :])
                    meng = nc.vector if pr < 2 else nc.gpsimd
                    for k in range(KC):
                        meng.tensor_mul(rhs8[pr][:, k, :], pT[pr][:, k, :],
                                        bc[:, :])

            prep_squares(0)
            prep_squares(1)
            prep_finish((0, 1))

            def main_phase(ph, preps=()):
                # 2 pairs per [128,2,512] PSUM tile (1 bank each, 392 cols).
                preps = dict(preps)
                prs = (2 * ph, 2 * ph + 1)
                for c in range(C):
                    ps = psum.tile([128, 2, 512], F32, tag="mm", bufs=2,
                                   name="mm")
                    for j in range(NKP):
                        for i, pr in enumerate(prs):
                            nc.tensor.matmul(ps[:, i, 0:COLS],
                                             lhsT=cT[j][:, :, c * 128:(c + 1) * 128],
                                             rhs=rhs8[pr][:, 2 * j:2 * j + 2, :],
                                             start=(j == 0), stop=(j == NKP - 1),
                                             perf_mode=DR)
                    out_ap = maxcol[:, c, 4 * ph:4 * ph + 4].rearrange(
                        "p (b s) -> p b s", s=2)
                    v = ps[:, :, 0:VCOLS].rearrange("p b (s x) -> p b s x", s=2)
                    if c % 4 == 1:
                        # bounce via ACT (it reads PSUM too) to offload DVE
                        cps = work.tile([128, 2, 2, NPATCH], BF, tag="cps",
                                        bufs=2)
                        nc.scalar.copy(out=cps[:], in_=v)
                        nc.vector.reduce_max(out=out_ap, in_=cps[:], axis=AX.X)
                    else:
                        nc.vector.reduce_max(out=out_ap, in_=v, axis=AX.X)
                    for fn in preps.get(c, ()):
                        fn()

            main_phase(0, preps={
                1: (lambda: prep_squares(2),),
                4: (lambda: prep_squares(3),),
                7: (lambda: prep_finish((2, 3)),),
            })

            # ---- concept row norms + G scaling (overlaps with main loop) ----
            for c in range(C):
                cn = work.tile([128, D], BF, tag="cnat", bufs=3)
                nc.sync.dma_start(out=cn[:], in_=d_cnat.ap()[c * 128:(c + 1) * 128, :])
                scr = work.tile([128, D], BF, tag="scr", bufs=3)
                nc.scalar.activation(out=scr[:], in_=cn[:], func=AF.Square,
                                     accum_out=css[:, c:c + 1])
            nc.scalar.sqrt(css[:], css[:])
            nc.vector.reciprocal(rnorm[:], css[:])
            for c in range(C):
                nc.sync.dma_start(out=GTbf[:, c * B:(c + 1) * B], in_=d_GT.ap()[c])
                nc.vector.tensor_scalar_mul(GTbf[:, c * B:(c + 1) * B],
                                            GTbf[:, c * B:(c + 1) * B],
                                            rnorm[:, c:c + 1])

            main_phase(1, preps={
                1: (lambda: prep_squares(4),),
                4: (lambda: prep_squares(5),),
                7: (lambda: prep_finish((4, 5)),),
            })

            # ---- IT-align: raw bf16 matmul + deferred rank-1 normalization --
            txtT = consts.tile([128, KC, 128], BF, tag="txtT")
            nc.sync.dma_start(out=txtT[:], in_=d_txtT.ap())
            imgT = consts.tile([128, KC, M_PER], BF, tag="imgT")
            nc.sync.dma_start(out=imgT[:], in_=d_imgT.ap())
            txtn = work.tile([128, D], BF, tag="cnat", bufs=3)
            nc.sync.dma_start(out=txtn[:], in_=d_txtn.ap())
            tscr = work.tile([128, D], BF, tag="scr", bufs=3)
            tss = small.tile([128, 1], F32, tag="tss")
            nc.scalar.activation(out=tscr[:], in_=txtn[:], func=AF.Square,
                                 accum_out=tss[:])
            nc.scalar.sqrt(tss[:], tss[:])
            av = small.tile([128, 1], F32, tag="av")
            nc.vector.reciprocal(av[:], tss[:])
            nc.vector.tensor_scalar_mul(av[:], av[:], float(t))  # t/||txt_v||

            # img norms via ones-matmul on squared imgT (transposed layout)
            isq = small.tile([128, KC, M_PER], BF, tag="isq")
            nc.scalar.activation(out=isq[:], in_=imgT[:], func=AF.Square)
            aux = psum.tile([128, 2, 512], F32, tag="aux", bufs=2, name="aux")
            for k in range(KC):
                nc.tensor.matmul(aux[0:1, 0, 0:M_PER], lhsT=ones_col[:, :],
                                 rhs=isq[:, k, :], start=(k == 0),
                                 stop=(k == KC - 1))
            ib = small.tile([1, M_PER], F32, tag="ib")
            nc.scalar.sqrt(ib[:], aux[0:1, 0, 0:M_PER])
            nc.vector.reciprocal(ib[:], ib[:])                   # 1/||img_m||
            bg = small.tile([128, M_PER], F32, tag="bg")
            nc.gpsimd.partition_broadcast(out_ap=bg[:, :], in_ap=ib[0:1, :])

            for k in range(KC):
                nc.tensor.matmul(aux[:, 1, 0:M_PER], lhsT=txtT[:, k, :],
                                 rhs=imgT[:, k, :], start=(k == 0),
                                 stop=(k == KC - 1))
            nc.scalar.activation(out=yit[:], in_=aux[:, 1, 0:M_PER], func=AF.Copy,
                                 scale=av[:])
            nc.vector.tensor_mul(yit[:], yit[:], bg[:])
            nc.vector.tensor_scalar_add(yit[:], yit[:], float(bias))

            nc.scalar.activation(out=warm[:], in_=warm[:], func=AF.Exp)

            main_phase(2, preps={
                1: (lambda: prep_squares(6),),
                4: (lambda: prep_squares(7),),
                7: (lambda: prep_finish((6, 7)),),
            })
            main_phase(3)

            # ---- S[v, m] = sum_p Geff[p,v] * maxcol[p,m]  (bf16, fp32 acc) --
            sps = psum.tile([128, 2, 512], F32, tag="aux", bufs=2, name="sps")
            for c in range(C):
                nc.tensor.matmul(sps[:, 0, 0:M_PER], lhsT=GTbf[:, c * B:(c + 1) * B],
                                 rhs=maxcol[:, c, :], start=(c == 0),
                                 stop=(c == C - 1))

            def softplus_out(y_ap, d_out):
                el = small.tile([B, M_PER], F32, tag="el", name="el")
                nc.scalar.activation(out=el[:], in_=y_ap, func=AF.Exp)
                nc.vector.tensor_scalar_add(el[:], el[:], 1.0)
                nc.scalar.activation(out=el[:], in_=el[:], func=AF.Ln)
                nc.sync.dma_start(out=d_out.ap(), in_=el[:])

            yrc = small.tile([B, M_PER], F32, tag="y")
            nc.scalar.activation(out=yrc[:], in_=sps[:, 0, 0:M_PER], func=AF.Copy,
                                 bias=float(bias), scale=float(t))
            nc.vector.tensor_mul(yrc[:], yrc[:], sign[:])
            softplus_out(yrc[:], d_rc)

            nc.vector.tensor_mul(yit[:], yit[:], sign[:])
            softplus_out(yit[:], d_it)

    nc.compile()
    return nc


def _install_trace_hook():
    """Register the axon NTFF profiling hook (missing from this image) so
    run_bass_kernel_spmd(trace=True) can capture HW exec time."""
    import contextlib
    import ctypes
    import types

    import concourse.bass_utils as bu

    if "antenv.axon_hooks" in sys.modules:
        return
    so_path = "/opt/axon/libaxon_pjrt.so"

    def _make_hook():
        lib = ctypes.CDLL(so_path)
        if not hasattr(lib, "axon_start_nrt_profile"):
            return None
        lib.axon_start_nrt_profile.argtypes = [ctypes.POINTER(ctypes.c_int64),
                                               ctypes.c_size_t]
        lib.axon_start_nrt_profile.restype = ctypes.c_int64
        lib.axon_stop_nrt_profile.argtypes = [ctypes.c_char_p]
        lib.axon_stop_nrt_profile.restype = ctypes.c_int64

        @contextlib.contextmanager
        def _hook(output_dir, device_ids):
            import jax
            jax.devices()
            if device_ids:
                ids = (ctypes.c_int64 * len(device_ids))(*device_ids)
                rc = lib.axon_start_nrt_profile(ids, len(device_ids))
            else:
                rc = lib.axon_start_nrt_profile(None, 0)
            if rc != 0:
                raise RuntimeError(f"axon_start_nrt_profile rc={rc}")
            try:
                yield
            finally:
                n = lib.axon_stop_nrt_profile(str(output_dir).encode())
                print(f"profile: {n} file(s) written to {output_dir}",
                      file=sys.stderr)

        return _hook

    mod = types.ModuleType("antenv.axon_hooks")
    mod.get_axon_ntff_profile_hook = _make_hook
    sys.modules["antenv.axon_hooks"] = mod
    bu.upload_artifacts = lambda tmpdir: tmpdir  # no S3 in this container


def _prepare(inputs):
    image_features = np.asarray(inputs["image_features"], np.float32)
    text_features = np.asarray(inputs["text_features"], np.float32)
    image_token_features = np.asarray(inputs["image_token_features"], np.float32)
    concept_text_features = np.asarray(inputs["concept_text_features"], np.float32)
    counts = np.asarray(inputs["concept_counts"]).astype(np.int64)
    t = float(np.exp(np.clip(np.float32(inputs["logit_scale"]), -10.0, 10.0)))
    bias = float(np.float32(inputs["logit_bias"]))

    # pack concepts: keep only w < counts[v]; pad rows with ones (zero weight)
    vidx = np.repeat(np.arange(B), counts)
    widx = np.concatenate([np.arange(c) for c in counts])
    P = len(vidx)
    C = math.ceil(P / 128)
    P2 = C * 128
    cnat = np.ones((P2, D), np.float32)
    cnat[:P] = concept_text_features[vidx, widx]
    cnat_bf = cnat.astype(BF16)
    # cT[j, d128, h, p] = fp8(cnat[p, (2j+h)*128 + d])
    cT = np.ascontiguousarray(
        cnat.astype(FP8).T.reshape(NKP, 2, 128, P2).transpose(0, 2, 1, 3))

    # G with 1/(16*counts): folds away the x16 patch scale
    G = np.zeros((P2, B), np.float32)
    G[np.arange(P), vidx] = 1.0 / (16.0 * counts[vidx])
    GT = G.astype(BF16).reshape(C, 128, B)

    txt_bf = text_features.astype(BF16)
    # txtT[d, k, v] = txt_bf[v, k*128 + d]
    txtT = np.ascontiguousarray(
        txt_bf.T.reshape(KC, 128, B).transpose(1, 0, 2))

    in_maps = []
    for core in range(N_CORES):
        s = slice(core * M_PER, (core + 1) * M_PER)
        sh = image_token_features[s].astype(BF16)        # (16, 196, 768)
        pT = np.zeros((PAIRS, 128, KC, COLS), BF16)
        shT = sh.transpose(0, 2, 1).reshape(M_PER, KC, 128, NPATCH)
        # pT[pr, d, k, 196*i2 + n] = patches[2pr+i2][n, k*128+d]
        pT[:, :, :, 0:NPATCH] = shT[0::2].transpose(0, 2, 1, 3)
        pT[:, :, :, NPATCH:VCOLS] = shT[1::2].transpose(0, 2, 1, 3)
        img_bf = image_features[s].astype(BF16)          # (16, 768)
        imgT = np.ascontiguousarray(
            img_bf.T.reshape(KC, 128, M_PER).transpose(1, 0, 2))
        signneg = np.ones((B, M_PER), np.float32)
        for j in range(M_PER):
            signneg[core * M_PER + j, j] = -1.0
        in_maps.append({
            "pT": pT,
            "cT": cT,
            "cnat": cnat_bf,
            "GT": GT,
            "txtT": txtT,
            "imgT": imgT,
            "txtn": txt_bf,
            "signneg": signneg,
        })
    return in_maps, C, t, bias


def _run(inputs, trace=False, tmpdir=None):
    in_maps, C, t, bias = _prepare(inputs)
    key = (C, t, bias)
    if key not in _cache:
        _cache[key] = _build(C, t, bias)
    nc = _cache[key]
    kwargs = {}
    if trace:
        _install_trace_hook()
        kwargs = dict(trace=True, tmpdir=tmpdir)
    res = run_bass_kernel_spmd(nc, in_maps, core_ids=list(range(N_CORES)),
                               **kwargs)
    it_sum = sum(float(r["it_el"].astype(np.float64).sum()) for r in res.results)
    rc_sum = sum(float(r["rc_el"].astype(np.float64).sum()) for r in res.results)
    it_loss = it_sum / (B * B)
    rc_loss = rc_sum / (B * B)
    total = it_loss + 0.5 * rc_loss
    out = (np.float32(total), np.float32(it_loss), np.float32(rc_loss))
    return out, res


def kernel(**inputs):
    out, _ = _run(inputs)
    return out
